# revision 6
# baseline (speedup 1.0000x reference)
"""MoE feed-forward (SwiGLU, top-2 routing, capacity 1.25) on 8 Trainium2 cores.

Expert parallelism: core d owns experts 2d and 2d+1. The host computes the
(tiny) gate + routing in numpy, builds per-expert dispatch buffers in
transposed layout [H, capk], and each core runs the three expert GEMMs
(gate/up proj + SwiGLU + down proj) in a Bass/Tile kernel with fp32r
(TF32-like) matmuls. The host then applies the weighted combine scatter.

The dispatch capacity is chosen adaptively: the reference capacity is
CAP=2560 slots/expert, but expert loads hover around 2050, so we compile
the kernel for the smallest capacity in a ladder that holds every expert's
actual token count. Any input fits some rung (the top rung is the full
reference capacity), so results are always exact w.r.t. the reference
routing semantics.

Self-contained: hardcodes all shapes; no sibling imports.
"""

import os

import numpy as np

# problem shapes
B, S, H, I, E, K = 8, 2048, 1024, 1024, 16, 2
N = B * S
CAP = 2560  # ceil(1.25 * N * K / E) — reference capacity
NCORES = 8
EL = E // NCORES  # experts per core
P = 128
HS = H // P  # h subtiles
IS_ = I // P  # i subtiles

# capacity ladder: smallest rung that holds max expert count gets used.
# chunk lists start with 256 (fast PE start) and keep every chunk >=256
# (fp32r needs moving-dim >=256 for full rate).
LADDER = [
    (2048, [512, 512, 512, 256, 256]),
    (2176, [512, 512, 512, 384, 256]),
    (2304, [512, 512, 512, 512, 256]),
    (2432, [512, 512, 512, 512, 384]),
    (2560, [512, 512, 512, 512, 512]),
]

USE_FP32 = os.environ.get("BASS_MOE_FP32", "0") == "1"

_NC_CACHE = {}


def _mybir():
    import concourse.mybir as mybir

    return mybir


def _build_nc(capk, chunks):
    import concourse.mybir as mybir
    import concourse.tile as tile
    from concourse import bacc

    assert sum(chunks) == capk
    f32 = mybir.dt.float32
    mm_dt = f32 if USE_FP32 else mybir.dt.float32r

    nc = bacc.Bacc("TRN2", target_bir_lowering=False, debug=False)
    xeT = nc.dram_tensor("xeT", [EL, H, capk], f32, kind="ExternalInput").ap()
    wgT = nc.dram_tensor("wgT", [EL, H, I], f32, kind="ExternalInput").ap()
    wuT = nc.dram_tensor("wuT", [EL, H, I], f32, kind="ExternalInput").ap()
    wdT = nc.dram_tensor("wdT", [EL, I, H], f32, kind="ExternalInput").ap()
    oeT = nc.dram_tensor("oeT", [EL, H, capk], f32, kind="ExternalOutput").ap()

    def cast(ap):
        return ap if USE_FP32 else ap.bitcast(mm_dt)

    MAXC = max(chunks)

    with tile.TileContext(nc) as tc:
        with (
            tc.tile_pool(name="wgp", bufs=9) as wgpool,
            tc.tile_pool(name="wup", bufs=9) as wupool,
            tc.tile_pool(name="wdp", bufs=9) as wdpool,
            tc.tile_pool(name="xin", bufs=2) as xpool,
            tc.tile_pool(name="hbuf", bufs=12) as hpool,
            tc.tile_pool(name="sig", bufs=2) as sigpool,
            tc.tile_pool(name="outs", bufs=3) as opool,
            tc.tile_pool(name="pg", bufs=2, space="PSUM") as pgpool,
            tc.tile_pool(name="pu", bufs=2, space="PSUM") as pupool,
            tc.tile_pool(name="po", bufs=2, space="PSUM") as popool,
        ):
            for e in range(EL):
                xeT_r = xeT[e].rearrange("(hs p) c -> p hs c", p=P)
                oeT_r = oeT[e].rearrange("(ns p) c -> p ns c", p=P)
                wgT_r = wgT[e].rearrange("(hs p) i -> p hs i", p=P)
                wuT_r = wuT[e].rearrange("(hs p) i -> p hs i", p=P)
                wdT_r = wdT[e].rearrange("(is p) n -> p is n", p=P)

                # chunk-0 activations first so the PE can start ASAP
                cs0 = chunks[0]
                x0 = xpool.tile([P, HS, MAXC], mm_dt, name="x")
                nc.sync.dma_start(x0[:, :, :cs0], cast(xeT_r[:, :, 0:cs0]))

                # stream gate/up weights in per-128-column tiles, first-use
                # order, on the ACT HWDGE ring (x/out use the SP ring)
                wg_t, wu_t, wd_t = [], [], []
                for it in range(IS_):
                    isl = slice(it * P, (it + 1) * P)
                    wgt = wgpool.tile([P, HS, P], mm_dt, name="wg")
                    nc.scalar.dma_start(wgt[:], cast(wgT_r[:, :, isl]))
                    wg_t.append(wgt)
                    wut = wupool.tile([P, HS, P], mm_dt, name="wu")
                    nc.scalar.dma_start(wut[:], cast(wuT_r[:, :, isl]))
                    wu_t.append(wut)

                def gemm12(x_sb, h_t, cs):
                    for it in range(IS_):
                        pg = pgpool.tile([P, MAXC], f32, name="pg")
                        pu = pupool.tile([P, MAXC], f32, name="pu")
                        for hs in range(HS):
                            nc.tensor.matmul(
                                pg[:, :cs],
                                wg_t[it][:, hs, :],
                                x_sb[:, hs, :cs],
                                start=(hs == 0),
                                stop=(hs == HS - 1),
                            )
                        for hs in range(HS):
                            nc.tensor.matmul(
                                pu[:, :cs],
                                wu_t[it][:, hs, :],
                                x_sb[:, hs, :cs],
                                start=(hs == 0),
                                stop=(hs == HS - 1),
                            )
                        sig = sigpool.tile([P, MAXC], f32, name="sig")
                        nc.scalar.activation(
                            sig[:, :cs],
                            pg[:, :cs],
                            _mybir().ActivationFunctionType.Silu,
                        )
                        nc.vector.tensor_mul(
                            h_t[it][:, :cs], sig[:, :cs], pu[:, :cs]
                        )

                def gemm3(h_t, c0, cs):
                    for nt in range(HS):
                        po = popool.tile([P, MAXC], f32, name="po")
                        for it in range(IS_):
                            nc.tensor.matmul(
                                po[:, :cs],
                                wd_t[nt][:, it, :],
                                h_t[it][:, :cs],
                                start=(it == 0),
                                stop=(it == IS_ - 1),
                            )
                        ot = opool.tile([P, MAXC], f32, name="ot")
                        nc.scalar.copy(ot[:, :cs], po[:, :cs])
                        nc.sync.dma_start(oeT_r[:, nt, c0 : c0 + cs], ot[:, :cs])

                def alloc_h():
                    return [hpool.tile([P, MAXC], mm_dt, name="h") for _ in range(IS_)]

                # chunk 0 GEMM1/2 while weights stream
                h0 = alloc_h()
                gemm12(x0, h0, cs0)

                # next chunk's x before the down-proj weights on the DMA ring
                if len(chunks) > 1:
                    cs1 = chunks[1]
                    x1 = xpool.tile([P, HS, MAXC], mm_dt, name="x")
                    nc.sync.dma_start(
                        x1[:, :, :cs1], cast(xeT_r[:, :, cs0 : cs0 + cs1])
                    )

                for nt in range(HS):
                    nsl = slice(nt * P, (nt + 1) * P)
                    wdt = wdpool.tile([P, IS_, P], mm_dt, name="wd")
                    nc.scalar.dma_start(wdt[:], cast(wdT_r[:, :, nsl]))
                    wd_t.append(wdt)

                gemm3(h0, 0, cs0)

                c0 = cs0
                for ci in range(1, len(chunks)):
                    cs = chunks[ci]
                    if ci == 1:
                        x_sb = x1
                    else:
                        x_sb = xpool.tile([P, HS, MAXC], mm_dt, name="x")
                        nc.sync.dma_start(
                            x_sb[:, :, :cs], cast(xeT_r[:, :, c0 : c0 + cs])
                        )
                    h_sb = alloc_h()
                    gemm12(x_sb, h_sb, cs)
                    gemm3(h_sb, c0, cs)
                    c0 += cs

    nc.compile()
    return nc


def _get_nc(capk):
    if capk not in _NC_CACHE:
        chunks = dict(LADDER)[capk]
        _NC_CACHE[capk] = _build_nc(capk, chunks)
    return _NC_CACHE[capk]


def host_route(xf, gate_w):
    """Numpy replica of the reference gating/capacity logic.

    Returns (tok_for_slot [E,CAP] int64, counts [E], combine per-expert
    (pos, tok, weight) lists). Only slots < count are meaningful.
    """
    logits = xf @ gate_w.T  # [N, E]
    m = logits.max(axis=-1, keepdims=True)
    ex = np.exp(logits - m)
    probs = ex / ex.sum(axis=-1, keepdims=True)
    topi = np.argsort(-probs, axis=-1, kind="stable")[:, :K].astype(np.int32)
    topw = np.take_along_axis(probs, topi, axis=-1)
    w = topw / np.clip(topw.sum(axis=-1, keepdims=True), 1e-9, None)

    flat_e = topi.reshape(-1)
    flat_w = w.reshape(-1).astype(np.float32)
    tok = np.repeat(np.arange(N, dtype=np.int64), K)

    perm = np.lexsort((-flat_w, flat_e))
    se, sw, st = flat_e[perm], flat_w[perm], tok[perm]
    counts = np.bincount(flat_e, minlength=E).astype(np.int64)
    starts = np.cumsum(counts) - counts
    pos = np.arange(N * K, dtype=np.int64) - starts[se]
    keep = pos < CAP  # reference semantics: drop beyond CAP per expert

    tok_for_slot = np.zeros((E, CAP), np.int64)
    ek, pk, tk, wk = se[keep], pos[keep], st[keep], sw[keep]
    tok_for_slot[ek, pk] = tk

    combine = []
    for ei in range(E):
        sel = ek == ei
        combine.append((pk[sel], tk[sel], wk[sel]))
    kept_counts = np.minimum(counts, CAP)
    return tok_for_slot, kept_counts, combine


def prepare(x, gate_w, wg, wu, wd):
    """Host routing + per-core input maps. Returns (in_maps, combine, capk)."""
    xf = np.ascontiguousarray(x.reshape(N, H), dtype=np.float32)
    tok_for_slot, counts, combine = host_route(xf, np.asarray(gate_w, np.float32))

    capk = LADDER[-1][0]
    for ck, _ in LADDER:
        if counts.max() <= ck:
            capk = ck
            break

    xfT = np.ascontiguousarray(xf.T)  # [H, N]
    wg = np.asarray(wg, np.float32)
    wu = np.asarray(wu, np.float32)
    wd = np.asarray(wd, np.float32)

    in_maps = []
    for d in range(NCORES):
        es = slice(EL * d, EL * (d + 1))
        xeT = np.empty((EL, H, capk), np.float32)
        for le in range(EL):
            xeT[le] = xfT[:, tok_for_slot[EL * d + le, :capk]]
        in_maps.append(
            {
                "xeT": xeT,
                "wgT": np.ascontiguousarray(wg[es].transpose(0, 2, 1)),
                "wuT": np.ascontiguousarray(wu[es].transpose(0, 2, 1)),
                "wdT": np.ascontiguousarray(wd[es].transpose(0, 2, 1)),
            }
        )
    return in_maps, combine, capk


def combine_results(results, combine):
    routedT = np.zeros((H, N), np.float32)
    for ei in range(E):
        d, le = divmod(ei, EL)
        Oe = results[d]["oeT"][le]  # [H, capk]
        pos_idx, tok_idx, wts = combine[ei]
        routedT[:, tok_idx] += Oe[:, pos_idx] * wts[None, :].astype(np.float32)
    return np.ascontiguousarray(routedT.T).reshape(B, S, H)


def kernel(x, gate_w, wg, wu, wd):
    from concourse.bass_utils import run_bass_kernel_spmd

    in_maps, combine, capk = prepare(x, gate_w, wg, wu, wd)
    nc = _get_nc(capk)
    res = run_bass_kernel_spmd(nc, in_maps, core_ids=list(range(NCORES)))
    routed = combine_results(res.results, combine)
    return routed, np.zeros((), np.float32)


# revision 7
# speedup vs baseline: 1.0877x; 1.0877x over previous
"""MoE feed-forward (SwiGLU, top-2 routing, capacity 1.25) on 8 Trainium2 cores.

Expert parallelism: core d owns experts 2d and 2d+1. The host computes the
(tiny) gate + routing in numpy, builds per-expert dispatch buffers in
transposed layout [H, capk], and each core runs the three expert GEMMs
(gate/up proj + SwiGLU + down proj) in a Bass/Tile kernel with fp32r
(TF32-like) matmuls. The host then applies the weighted combine scatter.

The dispatch capacity is chosen adaptively: the reference capacity is
CAP=2560 slots/expert, but expert loads hover around 2050, so we compile
the kernel for the smallest capacity in a ladder that holds every expert's
actual token count. Any input fits some rung (the top rung is the full
reference capacity), so results are always exact w.r.t. the reference
routing semantics.

Self-contained: hardcodes all shapes; no sibling imports.
"""

import os

import numpy as np

# problem shapes
B, S, H, I, E, K = 8, 2048, 1024, 1024, 16, 2
N = B * S
CAP = 2560  # ceil(1.25 * N * K / E) — reference capacity
NCORES = 8
EL = E // NCORES  # experts per core
P = 128
HS = H // P  # h subtiles
IS_ = I // P  # i subtiles

# capacity ladder: smallest rung that holds max expert count gets used.
# chunk lists start with 256 (fast PE start) and keep every chunk >=256
# (fp32r needs moving-dim >=256 for full rate).
LADDER = [
    (2048, [512, 512, 512, 256, 256]),
    (2176, [512, 512, 512, 384, 256]),
    (2304, [512, 512, 512, 512, 256]),
    (2432, [512, 512, 512, 512, 384]),
    (2560, [512, 512, 512, 512, 512]),
]

USE_FP32 = os.environ.get("BASS_MOE_FP32", "0") == "1"

_NC_CACHE = {}


def _mybir():
    import concourse.mybir as mybir

    return mybir


def _build_nc(capk, chunks):
    import concourse.mybir as mybir
    import concourse.tile as tile
    from concourse import bacc

    assert sum(chunks) == capk
    f32 = mybir.dt.float32
    mm_dt = f32 if USE_FP32 else mybir.dt.float32r

    nc = bacc.Bacc("TRN2", target_bir_lowering=False, debug=False)
    xeT = nc.dram_tensor("xeT", [EL, H, capk], f32, kind="ExternalInput").ap()
    wgT = nc.dram_tensor("wgT", [EL, H, I], f32, kind="ExternalInput").ap()
    wuT = nc.dram_tensor("wuT", [EL, H, I], f32, kind="ExternalInput").ap()
    wdT = nc.dram_tensor("wdT", [EL, I, H], f32, kind="ExternalInput").ap()
    oeT = nc.dram_tensor("oeT", [EL, H, capk], f32, kind="ExternalOutput").ap()

    def cast(ap):
        return ap if USE_FP32 else ap.bitcast(mm_dt)

    MAXC = max(chunks)

    with tile.TileContext(nc) as tc:
        with (
            tc.tile_pool(name="wgp", bufs=9) as wgpool,
            tc.tile_pool(name="wup", bufs=9) as wupool,
            tc.tile_pool(name="wdp", bufs=9) as wdpool,
            tc.tile_pool(name="xin", bufs=2) as xpool,
            tc.tile_pool(name="hbuf", bufs=12) as hpool,
            tc.tile_pool(name="sig", bufs=2) as sigpool,
            tc.tile_pool(name="outs", bufs=3) as opool,
            tc.tile_pool(name="pg", bufs=2, space="PSUM") as pgpool,
            tc.tile_pool(name="pu", bufs=2, space="PSUM") as pupool,
            tc.tile_pool(name="po", bufs=2, space="PSUM") as popool,
        ):
            for e in range(EL):
                xeT_r = xeT[e].rearrange("(hs p) c -> p hs c", p=P)
                oeT_r = oeT[e].rearrange("(ns p) c -> p ns c", p=P)
                wgT_r = wgT[e].rearrange("(hs p) i -> p hs i", p=P)
                wuT_r = wuT[e].rearrange("(hs p) i -> p hs i", p=P)
                wdT_r = wdT[e].rearrange("(is p) n -> p is n", p=P)

                # chunk-0 activations first so the PE can start ASAP
                cs0 = chunks[0]
                x0 = xpool.tile([P, HS, MAXC], mm_dt, name="x")
                nc.sync.dma_start(x0[:, :, :cs0], cast(xeT_r[:, :, 0:cs0]))

                # stream gate/up weights in per-128-column tiles, first-use
                # order, on the ACT HWDGE ring (x/out use the SP ring)
                wg_t, wu_t, wd_t = [], [], []
                for it in range(IS_):
                    isl = slice(it * P, (it + 1) * P)
                    wgt = wgpool.tile([P, HS, P], mm_dt, name="wg")
                    nc.sync.dma_start(wgt[:], cast(wgT_r[:, :, isl]))
                    wg_t.append(wgt)
                    wut = wupool.tile([P, HS, P], mm_dt, name="wu")
                    nc.sync.dma_start(wut[:], cast(wuT_r[:, :, isl]))
                    wu_t.append(wut)

                def gemm12(x_sb, h_t, cs):
                    for it in range(IS_):
                        pg = pgpool.tile([P, MAXC], f32, name="pg")
                        pu = pupool.tile([P, MAXC], f32, name="pu")
                        for hs in range(HS):
                            nc.tensor.matmul(
                                pg[:, :cs],
                                wg_t[it][:, hs, :],
                                x_sb[:, hs, :cs],
                                start=(hs == 0),
                                stop=(hs == HS - 1),
                            )
                        for hs in range(HS):
                            nc.tensor.matmul(
                                pu[:, :cs],
                                wu_t[it][:, hs, :],
                                x_sb[:, hs, :cs],
                                start=(hs == 0),
                                stop=(hs == HS - 1),
                            )
                        sig = sigpool.tile([P, MAXC], f32, name="sig")
                        nc.scalar.activation(
                            sig[:, :cs],
                            pg[:, :cs],
                            _mybir().ActivationFunctionType.Silu,
                        )
                        nc.vector.tensor_mul(
                            h_t[it][:, :cs], sig[:, :cs], pu[:, :cs]
                        )

                def gemm3(h_t, c0, cs):
                    for nt in range(HS):
                        po = popool.tile([P, MAXC], f32, name="po")
                        for it in range(IS_):
                            nc.tensor.matmul(
                                po[:, :cs],
                                wd_t[nt][:, it, :],
                                h_t[it][:, :cs],
                                start=(it == 0),
                                stop=(it == IS_ - 1),
                            )
                        ot = opool.tile([P, MAXC], f32, name="ot")
                        nc.scalar.copy(ot[:, :cs], po[:, :cs])
                        nc.sync.dma_start(oeT_r[:, nt, c0 : c0 + cs], ot[:, :cs])

                def alloc_h():
                    return [hpool.tile([P, MAXC], mm_dt, name="h") for _ in range(IS_)]

                # chunk 0 GEMM1/2 while weights stream
                h0 = alloc_h()
                gemm12(x0, h0, cs0)

                # next chunk's x before the down-proj weights on the DMA ring
                if len(chunks) > 1:
                    cs1 = chunks[1]
                    x1 = xpool.tile([P, HS, MAXC], mm_dt, name="x")
                    nc.sync.dma_start(
                        x1[:, :, :cs1], cast(xeT_r[:, :, cs0 : cs0 + cs1])
                    )

                for nt in range(HS):
                    nsl = slice(nt * P, (nt + 1) * P)
                    wdt = wdpool.tile([P, IS_, P], mm_dt, name="wd")
                    nc.sync.dma_start(wdt[:], cast(wdT_r[:, :, nsl]))
                    wd_t.append(wdt)

                gemm3(h0, 0, cs0)

                c0 = cs0
                for ci in range(1, len(chunks)):
                    cs = chunks[ci]
                    if ci == 1:
                        x_sb = x1
                    else:
                        x_sb = xpool.tile([P, HS, MAXC], mm_dt, name="x")
                        nc.sync.dma_start(
                            x_sb[:, :, :cs], cast(xeT_r[:, :, c0 : c0 + cs])
                        )
                    h_sb = alloc_h()
                    gemm12(x_sb, h_sb, cs)
                    gemm3(h_sb, c0, cs)
                    c0 += cs

    nc.compile()
    return nc


def _get_nc(capk):
    if capk not in _NC_CACHE:
        chunks = dict(LADDER)[capk]
        _NC_CACHE[capk] = _build_nc(capk, chunks)
    return _NC_CACHE[capk]


def host_route(xf, gate_w):
    """Numpy replica of the reference gating/capacity logic.

    Returns (tok_for_slot [E,CAP] int64, counts [E], combine per-expert
    (pos, tok, weight) lists). Only slots < count are meaningful.
    """
    logits = xf @ gate_w.T  # [N, E]
    m = logits.max(axis=-1, keepdims=True)
    ex = np.exp(logits - m)
    probs = ex / ex.sum(axis=-1, keepdims=True)
    topi = np.argsort(-probs, axis=-1, kind="stable")[:, :K].astype(np.int32)
    topw = np.take_along_axis(probs, topi, axis=-1)
    w = topw / np.clip(topw.sum(axis=-1, keepdims=True), 1e-9, None)

    flat_e = topi.reshape(-1)
    flat_w = w.reshape(-1).astype(np.float32)
    tok = np.repeat(np.arange(N, dtype=np.int64), K)

    perm = np.lexsort((-flat_w, flat_e))
    se, sw, st = flat_e[perm], flat_w[perm], tok[perm]
    counts = np.bincount(flat_e, minlength=E).astype(np.int64)
    starts = np.cumsum(counts) - counts
    pos = np.arange(N * K, dtype=np.int64) - starts[se]
    keep = pos < CAP  # reference semantics: drop beyond CAP per expert

    tok_for_slot = np.zeros((E, CAP), np.int64)
    ek, pk, tk, wk = se[keep], pos[keep], st[keep], sw[keep]
    tok_for_slot[ek, pk] = tk

    combine = []
    for ei in range(E):
        sel = ek == ei
        combine.append((pk[sel], tk[sel], wk[sel]))
    kept_counts = np.minimum(counts, CAP)
    return tok_for_slot, kept_counts, combine


def prepare(x, gate_w, wg, wu, wd):
    """Host routing + per-core input maps. Returns (in_maps, combine, capk)."""
    xf = np.ascontiguousarray(x.reshape(N, H), dtype=np.float32)
    tok_for_slot, counts, combine = host_route(xf, np.asarray(gate_w, np.float32))

    capk = LADDER[-1][0]
    for ck, _ in LADDER:
        if counts.max() <= ck:
            capk = ck
            break

    xfT = np.ascontiguousarray(xf.T)  # [H, N]
    wg = np.asarray(wg, np.float32)
    wu = np.asarray(wu, np.float32)
    wd = np.asarray(wd, np.float32)

    in_maps = []
    for d in range(NCORES):
        es = slice(EL * d, EL * (d + 1))
        xeT = np.empty((EL, H, capk), np.float32)
        for le in range(EL):
            xeT[le] = xfT[:, tok_for_slot[EL * d + le, :capk]]
        in_maps.append(
            {
                "xeT": xeT,
                "wgT": np.ascontiguousarray(wg[es].transpose(0, 2, 1)),
                "wuT": np.ascontiguousarray(wu[es].transpose(0, 2, 1)),
                "wdT": np.ascontiguousarray(wd[es].transpose(0, 2, 1)),
            }
        )
    return in_maps, combine, capk


def combine_results(results, combine):
    routedT = np.zeros((H, N), np.float32)
    for ei in range(E):
        d, le = divmod(ei, EL)
        Oe = results[d]["oeT"][le]  # [H, capk]
        pos_idx, tok_idx, wts = combine[ei]
        routedT[:, tok_idx] += Oe[:, pos_idx] * wts[None, :].astype(np.float32)
    return np.ascontiguousarray(routedT.T).reshape(B, S, H)


def kernel(x, gate_w, wg, wu, wd):
    from concourse.bass_utils import run_bass_kernel_spmd

    in_maps, combine, capk = prepare(x, gate_w, wg, wu, wd)
    nc = _get_nc(capk)
    res = run_bass_kernel_spmd(nc, in_maps, core_ids=list(range(NCORES)))
    routed = combine_results(res.results, combine)
    return routed, np.zeros((), np.float32)
